# revision 1
# baseline (speedup 1.0000x reference)
"""Bahdanau pointer-attention kernel for Trainium2 (8 NeuronCores, SPMD).

Computes energy[b, 1, n] = V . tanh(x[b, :N] @ W1.T + x[b, -1] @ W2.T)
for B=32, N=2048, D=1024.

Sharding: data-parallel over batch B across 8 cores (4 batches/core).
Per-core layout: contraction over d requires d on SBUF partitions, so the
host pre-transposes each core's x shard to [D, 4*N] during sharding.

Per-core pipeline (Tile framework):
  - keys matmul: psum[e128, n512] += W1T[d128, e128].T @ xT[d128, n512]
    (both operands float32r - 1 PE pass at full rate, ~fp22 precision)
  - ACT: tanh(psum + query_bias) fused via activation bias (per-partition)
  - V-dot: psum[1, n512] += VT[e128, 1].T @ tanh[e128, n512] on PE (bf16)
  - query preamble: psum[e128, b4] += W2T[d128, e128].T @ xqT[d128, b4]
"""

from contextlib import ExitStack

import numpy as np
import ml_dtypes

import concourse.bass as bass
import concourse.mybir as mybir
import concourse.tile as tile
from concourse import bacc
from concourse.bass_utils import run_bass_kernel_spmd

B, N, D = 32, 2048, 1024
CORES = 8
BPC = B // CORES            # batches per core
NTOT = BPC * N              # 8192 key positions per core
P = 128
DC = D // P                 # 8 d-chunks (contraction)
EC = D // P                 # 8 e-chunks (output feature)
NT = 512                    # n tile (one PSUM bank of f32)
NCH = NTOT // NT            # 16 n-chunks per core
NPB = N // NT               # n-chunks per batch

f32 = mybir.dt.float32
f32r = mybir.dt.float32r
bf16 = mybir.dt.bfloat16

TRACE = False
LAST_EXEC_NS = None
LAST_RESULTS = None

_NC_CACHE = {}


def _w_slice(w_sb, dc, ec):
    return w_sb[:, dc, ec * P:(ec + 1) * P]


def _body(ctx, tc, xT, xqT, w1T, w2T, vT, out, reps=1,
          do_dma=True, do_mm=True, do_act=True, do_vdot=True,
          split_dma=False, x_bufs=3, kpsum_bufs=3, group_n=1,
          same_w=False, xw_dt=f32r, vdot_pack=False):
    nc = tc.nc
    Tanh = mybir.ActivationFunctionType.Tanh

    w_pool = ctx.enter_context(tc.tile_pool(name="w", bufs=1))
    x_pool = ctx.enter_context(tc.tile_pool(name="x", bufs=x_bufs))
    t_pool = ctx.enter_context(
        tc.tile_pool(name="tanh", bufs=(2 * group_n + 1) * EC))
    small = ctx.enter_context(tc.tile_pool(name="small", bufs=1))
    en_pool = ctx.enter_context(tc.tile_pool(name="en", bufs=3))
    kpsum = ctx.enter_context(tc.tile_pool(name="kpsum", bufs=kpsum_bufs, space="PSUM"))
    vpsum = ctx.enter_context(tc.tile_pool(name="vpsum", bufs=2, space="PSUM"))
    qpsum = ctx.enter_context(tc.tile_pool(name="qpsum", bufs=2, space="PSUM"))

    # Resident weights, d-chunk on partitions: [p=128, (c, e)]
    w1_sb = w_pool.tile([P, DC, D], xw_dt, tag="w1")
    nc.sync.dma_start(w1_sb[:], w1T.rearrange("(c p) e -> p c e", p=P))
    w2_sb = w_pool.tile([P, DC, D], xw_dt, tag="w2")
    nc.sync.dma_start(w2_sb[:], w2T.rearrange("(c p) e -> p c e", p=P))
    v_sb = small.tile([P, EC], bf16, tag="v")
    nc.sync.dma_start(v_sb[:], vT[:, :])
    xq_sb = small.tile([P, DC, BPC], xw_dt, tag="xq")
    nc.sync.dma_start(xq_sb[:], xqT.rearrange("(c p) b -> p c b", p=P))

    # Query preamble: q_sb[e128, (ec, b)] = x_query @ W2.T  (transposed)
    q_sb = small.tile([P, EC * BPC], f32, tag="q")
    for ec in range(EC):
        pq = qpsum.tile([P, BPC], f32)
        for dc in range(DC):
            nc.tensor.matmul(
                pq[:],
                lhsT=_w_slice(w2_sb, dc, ec),
                rhs=xq_sb[:, dc, :],
                start=(dc == 0),
                stop=(dc == DC - 1),
            )
        nc.vector.tensor_copy(q_sb[:, ec * BPC:(ec + 1) * BPC], pq[:])

    # Main loop, software-pipelined: V-dot for chunk k-1 is emitted after
    # the keys matmuls of chunk k so the PE never waits on ACT.
    pending = []  # [(tanh tiles, chunk index), ...]
    x_fixed = None
    GN = group_n
    if not do_dma:
        x_fixed = x_pool.tile([P, DC, NT], xw_dt, tag="x")
        nc.sync.dma_start(
            x_fixed[:], xT.rearrange("(c p) n -> p c n", p=P)[:, :, 0:NT])
    for rep_g in range(reps * NCH // GN):
        chs = [(rep_g * GN + j) % NCH for j in range(GN)]
        xs = []
        for ch in chs:
            if do_dma:
                x_sb = x_pool.tile([P, DC, NT], xw_dt, tag="x")
                src = xT.rearrange("(c p) n -> p c n", p=P)[
                    :, :, ch * NT:(ch + 1) * NT]
                if split_dma:
                    nc.sync.dma_start(x_sb[:, :DC // 2, :], src[:, :DC // 2, :])
                    nc.scalar.dma_start(x_sb[:, DC // 2:, :], src[:, DC // 2:, :])
                else:
                    nc.sync.dma_start(x_sb[:], src)
            else:
                x_sb = x_fixed
            xs.append(x_sb)
        if not do_mm:
            continue
        ttsl = [[] for _ in chs]
        for ec in range(EC):
            pks = [kpsum.tile([P, NT], f32, tag="pk", name=f"pk{g}")
                   for g in range(GN)]
            for dc in range(DC):
                for g in range(GN):
                    nc.tensor.matmul(
                        pks[g][:],
                        lhsT=_w_slice(w1_sb, 0 if same_w else dc, ec),
                        rhs=xs[g][:, dc, :],
                        start=(dc == 0),
                        stop=(dc == DC - 1),
                    )
            if not do_act:
                continue
            for g, ch in enumerate(chs):
                b = ch // NPB
                tt = t_pool.tile([P, NT], bf16, tag="tanh")
                nc.scalar.activation(
                    tt[:], pks[g][:], Tanh,
                    bias=q_sb[:, ec * BPC + b: ec * BPC + b + 1],
                )
                ttsl[g].append(tt)
        if not (do_act and do_vdot):
            continue
        for p in pending:
            _emit_vdot(nc, vpsum, en_pool, v_sb, out, *p, pack=vdot_pack)
        pending = [(ttsl[g], chs[g]) for g in range(GN)]
    for p in pending:
        _emit_vdot(nc, vpsum, en_pool, v_sb, out, *p, pack=vdot_pack)


def _emit_vdot(nc, vpsum, en_pool, v_sb, out, tts, ch, pack=False):
    if not pack:
        pv = vpsum.tile([1, NT], f32)
        for ec in range(EC):
            nc.tensor.matmul(
                pv[:],
                lhsT=v_sb[:, ec:ec + 1],
                rhs=tts[ec][:],
                start=(ec == 0),
                stop=(ec == EC - 1),
            )
        en = en_pool.tile([1, NT], f32, tag="en")
        nc.vector.tensor_copy(en[:], pv[:])
        nc.sync.dma_start(out[:, ch * NT:(ch + 1) * NT], en[:])
        return
    # Packed: 4 concurrent col-groups (output partitions 0/32/64/96),
    # each accumulating 2 e-chunks; DVE sums the 4 partial rows.
    pv = vpsum.tile([P, NT], f32, name="pvp", tag="pvp")
    for ec in range(EC):
        j = ec % 4
        nc.tensor.matmul(
            pv[32 * j:32 * j + 1, :],
            lhsT=v_sb[:, ec:ec + 1],
            rhs=tts[ec][:],
            start=(ec < 4),
            stop=(ec >= 4),
            tile_position=(0, 32 * j),
            skip_group_check=True,
        )
    s0 = en_pool.tile([1, NT], f32, name="s0", tag="s0")
    s1 = en_pool.tile([1, NT], f32, name="s1", tag="s1")
    nc.vector.tensor_add(s0[:], pv[0:1, :], pv[32:33, :])
    nc.vector.tensor_add(s1[:], pv[64:65, :], pv[96:97, :])
    en = en_pool.tile([1, NT], f32, tag="en")
    nc.vector.tensor_add(en[:], s0[:], s1[:])
    nc.sync.dma_start(out[:, ch * NT:(ch + 1) * NT], en[:])


def build_module(reps=1, **opts):
    key = (reps, tuple(sorted(opts.items())))
    if key in _NC_CACHE:
        return _NC_CACHE[key]
    nc = bacc.Bacc("TRN2", target_bir_lowering=False, debug=False)
    xw_dt = opts.get("xw_dt", f32r)
    xT = nc.declare_dram_parameter("xT", [D, NTOT], xw_dt, isOutput=False)
    xqT = nc.declare_dram_parameter("xqT", [D, BPC], xw_dt, isOutput=False)
    w1T = nc.declare_dram_parameter("w1T", [D, D], xw_dt, isOutput=False)
    w2T = nc.declare_dram_parameter("w2T", [D, D], xw_dt, isOutput=False)
    vT = nc.declare_dram_parameter("vT", [P, EC], bf16, isOutput=False)
    out = nc.declare_dram_parameter("out", [1, NTOT], f32, isOutput=True)
    with tile.TileContext(nc) as tc:
        with ExitStack() as ctx:
            _body(ctx, tc, xT, xqT, w1T, w2T, vT, out, reps=reps, **opts)
    nc.compile()
    _NC_CACHE[key] = nc
    return nc


def shard_inputs(x, W1, W2, V, xw_bf16=False):
    """Host-side sharding + layout transforms. Returns per-core input maps."""
    x = np.asarray(x, dtype=np.float32)
    bf = ml_dtypes.bfloat16
    xdt = bf if xw_bf16 else np.float32
    w1T = np.ascontiguousarray(np.asarray(W1, np.float32).T).astype(xdt)
    w2T = np.ascontiguousarray(np.asarray(W2, np.float32).T).astype(xdt)
    vT = np.ascontiguousarray(np.asarray(V, np.float32).reshape(EC, P).T).astype(bf)
    in_maps = []
    for c in range(CORES):
        xs = x[c * BPC:(c + 1) * BPC, :N, :]          # [BPC, N, D]
        xT = np.ascontiguousarray(xs.transpose(2, 0, 1)).reshape(D, NTOT).astype(xdt)
        xq = x[c * BPC:(c + 1) * BPC, N, :]           # [BPC, D]
        xqT = np.ascontiguousarray(xq.T).astype(xdt)  # [D, BPC]
        in_maps.append({
            "xT": xT, "xqT": xqT,
            "w1T": w1T, "w2T": w2T, "vT": vT,
        })
    return in_maps


def kernel(x, W1, W2, V, city_count):
    global LAST_EXEC_NS, LAST_RESULTS
    assert int(city_count) == N
    nc = build_module()
    in_maps = shard_inputs(x, W1, W2, V)
    res = run_bass_kernel_spmd(nc, in_maps, core_ids=list(range(CORES)),
                               trace=TRACE)
    LAST_EXEC_NS = res.exec_time_ns
    LAST_RESULTS = res
    out = np.concatenate(
        [res.results[c]["out"].reshape(BPC, N) for c in range(CORES)], axis=0
    )
    return out[:, None, :].astype(np.float32)

